# revision 2
# baseline (speedup 1.0000x reference)
"""Mamba block Trainium2 kernel — 8-core SPMD, v2.

Sharding: tensor-parallel over d_inner (256 channels/core, both batches).
Per core: in_proj (PE) -> causal depthwise conv as 4 diag-matmuls (PE)
-> silu (ACT) -> x_proj partial + AllReduce -> dt/softplus (ACT) ->
selective scan: per (b,n) B/C rows broadcast to 128 partitions by
stride-0 DMA, decay a=exp(A_n*dt) on ACT (bf16), bb=u*B / ww=hh*C
split between DVE and the otherwise-idle GpSimd(Pool) engine, the
recurrence itself via DVE tensor_tensor_scan over the full 2048 steps,
y accumulated over n in PSUM by identity matmuls with D_skip folded in
as a diag-matmul -> gating (ACT copy + DVE bf16 mult) -> AllToAll to
token sharding -> out_proj. Host only shards/transposes/casts and
reassembles.
"""
import sys

sys.path.insert(0, "/opt/trn_rl_repo")

import numpy as np
import ml_dtypes

BF16 = np.dtype(ml_dtypes.bfloat16)

D_MODEL = 1024
D_STATE = 16
D_CONV = 4
D_INNER = 2048
BATCH = 2
SEQLEN = 2048
NCORES = 8
DLOC = D_INNER // NCORES          # 256 channels per core
ROWS = BATCH * DLOC               # 512 (b, d) rows per core
NRT = ROWS // 128                 # 4 row-tiles; rt -> (b=rt//2, i=rt%2)
TOK = BATCH * SEQLEN              # 4096 token columns, col = b*2048 + l
LT = SEQLEN // 4                  # 512-token chunks (PSUM bank size)
LC = SEQLEN // 2                  # 1024-col gating chunks

_cache = {}


def _split_sync_waits(nc, max_waits=1):
    """walrus in this toolchain accepts one sync-wait slot per instruction;
    move excess waits onto preceding same-engine NoOps."""
    import concourse.mybir as mb

    ctr = 0
    for fn in nc.m.functions:
        for bb in fn.blocks:
            out = []
            changed = False
            for inst in list(bb.instructions):
                si = inst.sync_info
                if si is not None and si.on_wait and len(si.on_wait) > max_waits:
                    waits = list(si.on_wait)
                    for w in waits[:-max_waits]:
                        ctr += 1
                        nop = mb.InstNoOp(name=f"syncsplit_nop_{ctr}", ins=[], outs=[])
                        nop.engine = inst.engine
                        nop.sync_info = mb.SyncInfo(on_wait=[w], on_update=[])
                        out.append(nop)
                    inst.sync_info = mb.SyncInfo(
                        on_wait=waits[-max_waits:],
                        on_update=list(si.on_update or []),
                    )
                    changed = True
                out.append(inst)
            if changed:
                bb.instructions.clear()
                bb.instructions.extend(out)
    return ctr


REPEATS = 1  # >1: unroll whole pipeline in-NEFF (timing builds)
# bb on Pool for n < bb_pool; ww on Pool for n < ww_pool (rest on DVE).
OPT = {"bb_pool": 16, "ww_pool": 6, "a_bf16": 1}


def _opt_key():
    return tuple(sorted(OPT.items()))


def _build_nc(repeats=None):
    import concourse.bass as bass
    import concourse.mybir as mybir
    from concourse.tile import TileContext

    f32 = mybir.dt.float32
    bf = mybir.dt.bfloat16
    AF = mybir.ActivationFunctionType
    OP = mybir.AluOpType

    reps = REPEATS if repeats is None else repeats
    nc = bass.Bass(num_devices=NCORES)

    def din(name, shape, dtype):
        return nc.dram_tensor(name, shape, dtype, kind="ExternalInput")

    hidT = din("hidT", [D_MODEL, TOK], bf)          # hidden^T, col = b*2048+l
    wxT = din("wxT", [D_MODEL, DLOC], bf)           # W_in x-rows slice, transposed
    wzT = din("wzT", [D_MODEL, DLOC], bf)           # W_in z-rows slice, transposed
    wpT = din("wpT", [DLOC, 33], bf)                # W_xproj columns slice, transposed
    woT = din("woT", [D_INNER, D_MODEL], bf)        # W_out^T (full)
    convd = din("convd", [NRT * D_CONV * 128, 128], bf)  # diag(conv_w) per (rt,k)
    dskd = din("dskd", [NRT * 128, 128], bf)        # diag(D_skip) per rt
    convb = din("convb", [ROWS, 1], f32)
    wdt = din("wdt", [ROWS, 1], f32)
    bdt = din("bdt", [ROWS, 1], f32)
    alog = din("alog", [ROWS, D_STATE], f32)
    ident_d = din("ident", [128, 128], bf)
    out_loc = nc.dram_tensor("out_loc", [D_MODEL, LT], f32, kind="ExternalOutput")

    xdbl_part = [nc.dram_tensor(f"xdbl_part{b}", [33, SEQLEN], f32)
                 for b in range(2)]
    xdbl_full = [nc.dram_tensor(f"xdbl_full{b}", [33, SEQLEN], f32,
                                addr_space="Shared") for b in range(2)]
    xdbl_bf = [nc.dram_tensor(f"xdbl_bf{b}", [33, SEQLEN], bf)
               for b in range(2)]
    a2a_in = nc.dram_tensor("a2a_in", [NCORES, DLOC, LT], bf)
    a2a_out = nc.dram_tensor("a2a_out", [NCORES, DLOC, LT], bf)

    a_dt = bf if OPT["a_bf16"] else f32

    with TileContext(nc) as tc:
        with (
            tc.tile_pool(name="pw", bufs=1) as pw,
            tc.tile_pool(name="pbig", bufs=1) as pbig,
        ):
            # ---- persistent weights / per-partition scalars ----
            wx = []
            wz = []
            for k in range(8):
                t = pw.tile([128, DLOC], bf, tag=f"wx{k}", name=f"wx{k}")
                nc.sync.dma_start(t[:], wxT[128 * k:128 * (k + 1), :])
                wx.append(t)
                t = pw.tile([128, DLOC], bf, tag=f"wz{k}", name=f"wz{k}")
                nc.sync.dma_start(t[:], wzT[128 * k:128 * (k + 1), :])
                wz.append(t)
            wxp = []
            for i in range(2):
                t = pw.tile([128, 33], bf, tag=f"wxp{i}", name=f"wxp{i}")
                nc.sync.dma_start(t[:], wpT[128 * i:128 * (i + 1), :])
                wxp.append(t)
            ident = pw.tile([128, 128], bf, tag="ident", name="ident")
            nc.sync.dma_start(ident[:], ident_d[:])
            cd = []
            for j in range(NRT * D_CONV):
                t = pw.tile([128, 128], bf, tag=f"cd{j}", name=f"cd{j}")
                nc.sync.dma_start(t[:], convd[128 * j:128 * (j + 1), :])
                cd.append(t)
            dsk = []
            for rt in range(NRT):
                t = pw.tile([128, 128], bf, tag=f"dsk{rt}", name=f"dsk{rt}")
                nc.sync.dma_start(t[:], dskd[128 * rt:128 * (rt + 1), :])
                dsk.append(t)

            cb, wd, bd, At = [], [], [], []
            for rt in range(NRT):
                rs = slice(128 * rt, 128 * (rt + 1))
                t = pw.tile([128, 1], f32, tag=f"cb{rt}", name=f"cb{rt}")
                nc.sync.dma_start(t[:], convb[rs, :])
                cb.append(t)
                t = pw.tile([128, 1], f32, tag=f"wd{rt}", name=f"wd{rt}")
                nc.sync.dma_start(t[:], wdt[rs, :])
                wd.append(t)
                t = pw.tile([128, 1], f32, tag=f"bd{rt}", name=f"bd{rt}")
                nc.sync.dma_start(t[:], bdt[rs, :])
                bd.append(t)
                ta = pw.tile([128, D_STATE], f32, tag=f"alog{rt}", name=f"alog{rt}")
                nc.sync.dma_start(ta[:], alog[rs, :])
                te = pw.tile([128, D_STATE], f32, tag=f"aexp{rt}", name=f"aexp{rt}")
                nc.scalar.activation(te[:], ta[:], AF.Exp)
                tn = pw.tile([128, D_STATE], f32, tag=f"aneg{rt}", name=f"aneg{rt}")
                nc.vector.tensor_scalar(tn[:], te[:], -1.0, None, OP.mult)
                At.append(tn)

            # ---- persistent activations ----
            x = [pbig.tile([128, SEQLEN], bf, tag=f"x{rt}", name=f"x{rt}") for rt in range(NRT)]
            z = [pbig.tile([128, SEQLEN], bf, tag=f"z{rt}", name=f"z{rt}") for rt in range(NRT)]
            dt_ = [pbig.tile([128, SEQLEN], f32, tag=f"dt{rt}", name=f"dt{rt}") for rt in range(NRT)]
            u = [pbig.tile([128, SEQLEN], bf, tag=f"u{rt}", name=f"u{rt}") for rt in range(NRT)]

            for _rep in range(reps):
                # ============ S1: in_proj + conv(+silu) + silu(z) ============
                with (
                    tc.tile_pool(name="ps1", bufs=1) as ps1,
                    tc.tile_pool(name="ph", bufs=2) as ph,
                    tc.tile_pool(name="pp1", bufs=2, space="PSUM") as pp1,
                ):
                    x_pre = [
                        ps1.tile([128, SEQLEN + 3], bf, tag=f"xp{rt}", name=f"xp{rt}")
                        for rt in range(NRT)
                    ]
                    for rt in range(NRT):
                        nc.vector.memset(x_pre[rt][:, 0:3], 0.0)
                    for b in range(2):
                        for c2 in range(2):
                            cs = slice(b * SEQLEN + LC * c2, b * SEQLEN + LC * (c2 + 1))
                            ht = []
                            for k in range(8):
                                t = ph.tile([128, LC], bf, tag=f"hid{k}", name=f"hid{k}")
                                nc.sync.dma_start(t[:], hidT[128 * k:128 * (k + 1), cs])
                                ht.append(t)
                            for c4 in range(2):
                                hs = slice(LT * c4, LT * (c4 + 1))
                                for i in range(2):
                                    rt = 2 * b + i
                                    ms = slice(128 * i, 128 * (i + 1))
                                    px = pp1.tile([128, LT], f32, tag="px", name="px")
                                    pz = pp1.tile([128, LT], f32, tag="pz", name="pz")
                                    for k in range(8):
                                        nc.tensor.matmul(px[:], wx[k][:, ms], ht[k][:, hs],
                                                         start=(k == 0), stop=(k == 7))
                                    for k in range(8):
                                        nc.tensor.matmul(pz[:], wz[k][:, ms], ht[k][:, hs],
                                                         start=(k == 0), stop=(k == 7))
                                    t0 = LC * c2 + LT * c4
                                    nc.scalar.copy(x_pre[rt][:, 3 + t0:3 + t0 + LT], px[:])
                                    nc.scalar.activation(
                                        z[rt][:, t0:t0 + LT], pz[:], AF.Silu)
                    # conv: 4 accumulating diag-matmuls per (rt, 512-chunk)
                    for rt in range(NRT):
                        for c4 in range(4):
                            pc = pp1.tile([128, LT], f32, tag="pc", name="pc")
                            for k in range(D_CONV):
                                nc.tensor.matmul(
                                    pc[:], cd[rt * D_CONV + k][:],
                                    x_pre[rt][:, LT * c4 + k:LT * c4 + k + LT],
                                    start=(k == 0), stop=(k == 3))
                            nc.scalar.activation(
                                x[rt][:, LT * c4:LT * (c4 + 1)], pc[:], AF.Silu,
                                bias=cb[rt][:])

                # ============ S2: x_proj + AllReduce + dt/softplus ============
                with (
                    tc.tile_pool(name="pxp", bufs=2) as pxp,
                    tc.tile_pool(name="pp3", bufs=2, space="PSUM") as pp3,
                ):
                    for b in range(2):
                        xdsb = pxp.tile([33, SEQLEN], f32, tag="xdsb", name=f"xdsb{b}")
                        for c4 in range(4):
                            ls = slice(LT * c4, LT * (c4 + 1))
                            p33 = pp3.tile([33, LT], f32, tag="p33", name="p33")
                            nc.tensor.matmul(p33[:], wxp[0][:], x[2 * b][:, ls],
                                             start=True, stop=False)
                            nc.tensor.matmul(p33[:], wxp[1][:], x[2 * b + 1][:, ls],
                                             start=False, stop=True)
                            nc.scalar.copy(xdsb[:, ls], p33[:])
                        nc.sync.dma_start(xdbl_part[b][:], xdsb[:])
                        nc.gpsimd.collective_compute(
                            "AllReduce", OP.add,
                            ins=[xdbl_part[b][:]], outs=[xdbl_full[b][:]],
                            replica_groups=[list(range(NCORES))],
                        )
                        xdsf = pxp.tile([33, SEQLEN], f32, tag="xdsf", name=f"xdsf{b}")
                        nc.sync.dma_start(xdsf[:], xdbl_full[b][:])
                        bcr = pxp.tile([33, SEQLEN], bf, tag="bcr", name=f"bcr{b}")
                        nc.vector.tensor_copy(bcr[:], xdsf[:, :])
                        nc.sync.dma_start(xdbl_bf[b][:], bcr[:])

                        # dt = softplus(W_dt*dt_raw + b_dt); u = dt*x
                        dtb_t = pxp.tile([128, SEQLEN], f32, tag="dtraw",
                                         name=f"dtraw{b}")
                        nc.sync.dma_start(
                            dtb_t[:],
                            xdbl_full[b][0:1, :].to_broadcast((128, SEQLEN)))
                        for i in range(2):
                            rt = 2 * b + i
                            # softplus(v) = ln(exp(v) + 1); |v| stays far from
                            # fp32 overflow for these inputs.
                            e1 = pxp.tile([128, SEQLEN], f32, tag="spe", name="spe")
                            nc.scalar.activation(e1[:], dtb_t[:], AF.Exp,
                                                 scale=wd[rt][:], bias=bd[rt][:])
                            nc.scalar.activation(dt_[rt][:], e1[:], AF.Ln, bias=1.0)
                            nc.vector.tensor_tensor(u[rt][:], dt_[rt][:], x[rt][:],
                                                    OP.mult)

                # ============ S3: selective scan + gating ============
                with (
                    tc.tile_pool(name="psc", bufs=2) as psc,
                    tc.tile_pool(name="pps", bufs=1, space="PSUM") as pps,
                ):
                    for b in range(2):
                        py = {}
                        for i in range(2):
                            py[i] = pps.tile([128, SEQLEN], f32,
                                             tag=f"py{i}", name=f"py{i}")
                        for n in range(16):
                            Bb = psc.tile([128, SEQLEN], bf, tag="Bb", name="Bb",
                                          bufs=3)
                            nc.sync.dma_start(
                                Bb[:],
                                xdbl_bf[b][1 + n:2 + n, :].to_broadcast(
                                    (128, SEQLEN)))
                            Cb = psc.tile([128, SEQLEN], bf, tag="Cb", name="Cb",
                                          bufs=3)
                            nc.sync.dma_start(
                                Cb[:],
                                xdbl_bf[b][17 + n:18 + n, :].to_broadcast(
                                    (128, SEQLEN)))
                            for i in range(2):
                                rt = 2 * b + i
                                a_t = psc.tile([128, SEQLEN], a_dt, tag="a",
                                               name="a", bufs=3)
                                nc.scalar.activation(
                                    a_t[:], dt_[rt][:], AF.Exp,
                                    scale=At[rt][:, n:n + 1])
                                bb = psc.tile([128, SEQLEN], bf, tag="bb",
                                              name="bb")
                                if n < OPT["bb_pool"]:
                                    nc.gpsimd.tensor_tensor(
                                        bb[:], u[rt][:], Bb[:], OP.mult)
                                else:
                                    nc.vector.tensor_tensor(
                                        bb[:], u[rt][:], Bb[:], OP.mult)
                                hh = psc.tile([128, SEQLEN], bf, tag="hh",
                                              name="hh")
                                nc.vector.tensor_tensor_scan(
                                    hh[:], a_t[:], bb[:], 0.0,
                                    OP.mult, OP.add)
                                ww = psc.tile([128, SEQLEN], bf, tag="ww",
                                              name="ww")
                                if n < OPT["ww_pool"]:
                                    nc.gpsimd.tensor_tensor(
                                        ww[:], hh[:], Cb[:], OP.mult)
                                else:
                                    nc.vector.tensor_tensor(
                                        ww[:], hh[:], Cb[:], OP.mult)
                                for q in range(4):
                                    nc.tensor.matmul(
                                        py[i][:, LT * q:LT * (q + 1)],
                                        ident[:],
                                        ww[:, LT * q:LT * (q + 1)],
                                        start=(n == 0), stop=False)
                        # D_skip term closes each accumulation group
                        for i in range(2):
                            rt = 2 * b + i
                            for q in range(4):
                                nc.tensor.matmul(
                                    py[i][:, LT * q:LT * (q + 1)],
                                    dsk[rt][:],
                                    x[rt][:, LT * q:LT * (q + 1)],
                                    start=False, stop=True)
                        # gating for this b
                        for i in range(2):
                            rt = 2 * b + i
                            for c2 in range(2):
                                gs = slice(LC * c2, LC * (c2 + 1))
                                ysc = psc.tile([128, LC], bf, tag="ysc",
                                               name="ysc")
                                nc.scalar.copy(ysc[:], py[i][:, gs])
                                yg = psc.tile([128, LC], bf, tag="yg",
                                              name="yg")
                                nc.vector.tensor_tensor(
                                    yg[:], ysc[:], z[rt][:, gs], OP.mult)
                                for hf in range(2):
                                    j = 4 * b + 2 * c2 + hf
                                    nc.sync.dma_start(
                                        a2a_in[j, 128 * i:128 * (i + 1), :],
                                        yg[:, LT * hf:LT * (hf + 1)])

                # ================= S8: AllToAll + out_proj =================
                nc.gpsimd.collective_compute(
                    "AllToAll", OP.bypass,
                    ins=[a2a_in[:]], outs=[a2a_out[:]],
                    replica_groups=[list(range(NCORES))],
                )
                with (
                    tc.tile_pool(name="po", bufs=1) as po,
                    tc.tile_pool(name="po2", bufs=2) as po2,
                    tc.tile_pool(name="ppo", bufs=2, space="PSUM") as ppo,
                ):
                    wo = []
                    for kt in range(16):
                        t = po.tile([128, D_MODEL], bf, tag=f"wo{kt}", name=f"wo{kt}")
                        nc.sync.dma_start(t[:], woT[128 * kt:128 * (kt + 1), :])
                        wo.append(t)
                    yf = []
                    for kt in range(16):
                        t = po.tile([128, LT], bf, tag=f"yf{kt}", name=f"yf{kt}")
                        nc.sync.dma_start(
                            t[:], a2a_out[kt // 2, 128 * (kt % 2):128 * (kt % 2 + 1), :])
                        yf.append(t)
                    for mt in range(8):
                        pO = ppo.tile([128, LT], f32, tag="pO", name="pO")
                        for kt in range(16):
                            nc.tensor.matmul(pO[:], wo[kt][:, 128 * mt:128 * (mt + 1)],
                                             yf[kt][:], start=(kt == 0), stop=(kt == 15))
                        osb = po2.tile([128, LT], f32, tag="osb", name="osb")
                        nc.scalar.copy(osb[:], pO[:])
                        nc.sync.dma_start(out_loc[128 * mt:128 * (mt + 1), :], osb[:])

    _split_sync_waits(nc)
    return nc


def _prep_inputs(inputs):
    hidden = np.ascontiguousarray(inputs["hidden_states"], np.float32)
    W_in = np.asarray(inputs["W_in"], np.float32)
    conv_w = np.asarray(inputs["conv_w"], np.float32)
    conv_b = np.asarray(inputs["conv_b"], np.float32)
    W_xproj = np.asarray(inputs["W_xproj"], np.float32)
    W_dt = np.asarray(inputs["W_dt"], np.float32)
    b_dt = np.asarray(inputs["b_dt"], np.float32)
    A_log = np.asarray(inputs["A_log"], np.float32)
    D_skip = np.asarray(inputs["D_skip"], np.float32)
    W_out = np.asarray(inputs["W_out"], np.float32)

    # hidden^T: [d_model, b*L]
    hidT = np.ascontiguousarray(
        hidden.transpose(2, 0, 1).reshape(D_MODEL, TOK)).astype(BF16)
    woT = np.ascontiguousarray(W_out.T).astype(BF16)
    ident = np.eye(128, dtype=np.float32).astype(BF16)

    in_maps = []
    for c in range(NCORES):
        dsl = slice(DLOC * c, DLOC * (c + 1))
        dup = lambda v: np.ascontiguousarray(
            np.tile(v[dsl].reshape(DLOC, -1), (BATCH, 1)), ).astype(np.float32)
        cwl = dup(conv_w[:, 0, :])                      # [ROWS, 4]
        convd = np.zeros((NRT * D_CONV, 128, 128), np.float32)
        for rt in range(NRT):
            for k in range(D_CONV):
                np.fill_diagonal(convd[rt * D_CONV + k],
                                 cwl[128 * rt:128 * (rt + 1), k])
        dskl = dup(D_skip)                              # [ROWS, 1]
        dskd = np.zeros((NRT, 128, 128), np.float32)
        for rt in range(NRT):
            np.fill_diagonal(dskd[rt], dskl[128 * rt:128 * (rt + 1), 0])
        in_maps.append(dict(
            hidT=hidT,
            wxT=np.ascontiguousarray(W_in[dsl, :].T).astype(BF16),
            wzT=np.ascontiguousarray(W_in[D_INNER:, :][dsl, :].T).astype(BF16),
            wpT=np.ascontiguousarray(W_xproj[:, dsl].T).astype(BF16),
            woT=woT,
            convd=np.ascontiguousarray(
                convd.reshape(NRT * D_CONV * 128, 128)).astype(BF16),
            dskd=np.ascontiguousarray(
                dskd.reshape(NRT * 128, 128)).astype(BF16),
            convb=dup(conv_b),
            wdt=dup(W_dt),
            bdt=dup(b_dt),
            alog=dup(A_log),
            ident=ident,
        ))
    return in_maps


def _gather(results):
    out = np.empty((BATCH, SEQLEN, D_MODEL), np.float32)
    for c in range(NCORES):
        b, lc = c // 4, c % 4
        out[b, LT * lc:LT * (lc + 1), :] = results[c]["out_loc"].T
    return out


def _get_nc(reps=None):
    key = ("nc", REPEATS if reps is None else reps, _opt_key())
    if key not in _cache:
        _cache[key] = _build_nc(repeats=reps)
    return _cache[key]


def _get_runner(reps=None, donate_outs=True):
    key = ("runner", REPEATS if reps is None else reps, donate_outs, _opt_key())
    nc = _get_nc(reps)
    if key not in _cache:
        from concourse import bass2jax
        import jax
        import concourse.mybir as mybir
        from jax.sharding import Mesh, PartitionSpec
        from jax.experimental.shard_map import shard_map

        bass2jax.install_neuronx_cc_hook()
        partition_name = (nc.partition_id_tensor.name
                          if nc.partition_id_tensor else None)
        in_names, out_names, out_avals, zero_outs = [], [], [], []
        for alloc in nc.m.functions[0].allocations:
            if not isinstance(alloc, mybir.MemoryLocationSet):
                continue
            name = alloc.memorylocations[0].name
            if alloc.kind == "ExternalInput":
                if name != partition_name:
                    in_names.append(name)
            elif alloc.kind == "ExternalOutput":
                out_names.append(name)
                shape = tuple(alloc.tensor_shape)
                dtype = mybir.dt.np(alloc.dtype)
                out_avals.append(jax.core.ShapedArray(shape, dtype))
                zero_outs.append(np.zeros(shape, dtype))
        n_params = len(in_names)
        all_in = in_names + out_names
        if partition_name is not None:
            all_in = all_in + [partition_name]
        donate = tuple(range(n_params, n_params + len(out_names)))

        def _body(*args):
            operands = list(args)
            if partition_name is not None:
                operands.append(bass2jax.partition_id_tensor())
            outs = bass2jax._bass_exec_p.bind(
                *operands,
                out_avals=tuple(out_avals),
                in_names=tuple(all_in),
                out_names=tuple(out_names),
                lowering_input_output_aliases=(),
                sim_require_finite=True,
                sim_require_nnan=True,
                nc=nc,
            )
            return tuple(outs)

        devices = jax.devices()[:NCORES]
        mesh = Mesh(np.asarray(devices), ("core",))
        in_specs = (PartitionSpec("core"),) * (n_params + len(out_names))
        out_specs = (PartitionSpec("core"),) * len(out_names)
        sharded = jax.jit(
            shard_map(_body, mesh=mesh, in_specs=in_specs, out_specs=out_specs,
                      check_rep=False),
            donate_argnums=(donate if donate_outs else ()), keep_unused=True)
        _cache[key] = (sharded, in_names, out_names, zero_outs, n_params, mesh)
    return _cache[key]


def device_put_inputs(in_maps, reps=None, donate_outs=True):
    """Concatenate per-core inputs and place them sharded on the devices."""
    import jax
    from jax.sharding import NamedSharding, PartitionSpec

    sharded, in_names, out_names, zero_outs, n_params, mesh = _get_runner(
        reps, donate_outs)
    sh = NamedSharding(mesh, PartitionSpec("core"))
    concat_in = [
        np.concatenate([np.asarray(in_maps[c][nm]) for c in range(NCORES)], axis=0)
        for nm in in_names
    ] + [np.concatenate([z] * NCORES, axis=0) for z in zero_outs]
    return [jax.device_put(a, sh) for a in concat_in]


def run_spmd(in_maps=None, dev_in=None, reps=None, donate_outs=True,
             as_numpy=True):
    """Execute on cores 0-7. First call compiles; later calls reuse the
    jitted executable."""
    sharded, in_names, out_names, zero_outs, n_params, mesh = _get_runner(
        reps, donate_outs)
    if dev_in is None:
        dev_in = device_put_inputs(in_maps, reps, donate_outs)
    outs = sharded(*dev_in)
    if not as_numpy:
        return outs
    outs = [np.asarray(o) for o in outs]
    results = []
    for c in range(NCORES):
        d = {}
        for i, nm in enumerate(out_names):
            per = outs[i].shape[0] // NCORES
            d[nm] = outs[i][per * c:per * (c + 1)]
        results.append(d)
    return results


def kernel(**inputs) -> np.ndarray:
    in_maps = _prep_inputs(inputs)
    results = run_spmd(in_maps)
    return _gather(results)


# revision 7
# speedup vs baseline: 1.6569x; 1.6569x over previous
"""Mamba block Trainium2 kernel — 8-core SPMD, v2.

Sharding: tensor-parallel over d_inner (256 channels/core, both batches).
Per core: in_proj (PE) -> causal depthwise conv as 4 diag-matmuls (PE)
-> silu (ACT) -> x_proj partial + AllReduce -> dt/softplus (ACT) ->
selective scan: per (b,n) B/C rows broadcast to 128 partitions by
stride-0 DMA, decay a=exp(A_n*dt) on ACT (bf16), bb=u*B / ww=hh*C
split between DVE and the otherwise-idle GpSimd(Pool) engine, the
recurrence itself via DVE tensor_tensor_scan over the full 2048 steps,
y accumulated over n in PSUM by identity matmuls with D_skip folded in
as a diag-matmul -> gating (ACT copy + DVE bf16 mult) -> AllToAll to
token sharding -> out_proj. Host only shards/transposes/casts and
reassembles.
"""
import sys

sys.path.insert(0, "/opt/trn_rl_repo")

import numpy as np
import ml_dtypes

BF16 = np.dtype(ml_dtypes.bfloat16)

D_MODEL = 1024
D_STATE = 16
D_CONV = 4
D_INNER = 2048
BATCH = 2
SEQLEN = 2048
NCORES = 8
DLOC = D_INNER // NCORES          # 256 channels per core
ROWS = BATCH * DLOC               # 512 (b, d) rows per core
NRT = ROWS // 128                 # 4 row-tiles; rt -> (b=rt//2, i=rt%2)
TOK = BATCH * SEQLEN              # 4096 token columns, col = b*2048 + l
LT = SEQLEN // 4                  # 512-token chunks (PSUM bank size)
LC = SEQLEN // 2                  # 1024-col gating chunks

_cache = {}


def _split_sync_waits(nc, max_waits=1):
    """walrus in this toolchain accepts one sync-wait slot per instruction;
    move excess waits onto preceding same-engine NoOps."""
    import concourse.mybir as mb

    ctr = 0
    for fn in nc.m.functions:
        for bb in fn.blocks:
            out = []
            changed = False
            for inst in list(bb.instructions):
                si = inst.sync_info
                if si is not None and si.on_wait and len(si.on_wait) > max_waits:
                    waits = list(si.on_wait)
                    for w in waits[:-max_waits]:
                        ctr += 1
                        nop = mb.InstNoOp(name=f"syncsplit_nop_{ctr}", ins=[], outs=[])
                        nop.engine = inst.engine
                        nop.sync_info = mb.SyncInfo(on_wait=[w], on_update=[])
                        out.append(nop)
                    inst.sync_info = mb.SyncInfo(
                        on_wait=waits[-max_waits:],
                        on_update=list(si.on_update or []),
                    )
                    changed = True
                out.append(inst)
            if changed:
                bb.instructions.clear()
                bb.instructions.extend(out)
    return ctr


REPEATS = 1  # >1: unroll whole pipeline in-NEFF (timing builds)
# bb on Pool for n < bb_pool; ww on Pool for n < ww_pool (rest on DVE).
OPT = {"bb_pool": 0, "ww_pool": 0, "a_bf16": 1, "ar_bf16": 1,
       "ht_bufs": 2}


def _opt_key():
    return tuple(sorted(OPT.items()))


def _build_nc(repeats=None):
    import concourse.bass as bass
    import concourse.mybir as mybir
    from concourse.tile import TileContext

    f32 = mybir.dt.float32
    bf = mybir.dt.bfloat16
    AF = mybir.ActivationFunctionType
    OP = mybir.AluOpType

    reps = REPEATS if repeats is None else repeats
    nc = bass.Bass(num_devices=NCORES)

    def din(name, shape, dtype):
        return nc.dram_tensor(name, shape, dtype, kind="ExternalInput")

    hidT = din("hidT", [D_MODEL, TOK], bf)          # hidden^T, col = b*2048+l
    wxT = din("wxT", [D_MODEL, DLOC], bf)           # W_in x-rows slice, transposed
    wzT = din("wzT", [D_MODEL, DLOC], bf)           # W_in z-rows slice, transposed
    wpT = din("wpT", [DLOC, 33], bf)                # W_xproj columns slice, transposed
    woT = din("woT", [D_INNER, D_MODEL], bf)        # W_out^T (full)
    convd = din("convd", [NRT * D_CONV * 128, 128], bf)  # diag(conv_w) per (rt,k)
    dskd = din("dskd", [NRT * 128, 128], bf)        # diag(D_skip) per rt
    convb = din("convb", [ROWS, 1], f32)
    wdt = din("wdt", [ROWS, 1], f32)
    bdt = din("bdt", [ROWS, 1], f32)
    alog = din("alog", [ROWS, D_STATE], f32)
    ident_d = din("ident", [128, 128], bf)
    out_loc = nc.dram_tensor("out_loc", [D_MODEL, LT], f32, kind="ExternalOutput")

    ar_dt = bf if OPT["ar_bf16"] else f32
    xdbl_part = [nc.dram_tensor(f"xdbl_part{b}", [33, SEQLEN], ar_dt)
                 for b in range(2)]
    xdbl_full = [nc.dram_tensor(f"xdbl_full{b}", [33, SEQLEN], ar_dt,
                                addr_space="Shared") for b in range(2)]
    xdbl_bf = [nc.dram_tensor(f"xdbl_bf{b}", [33, SEQLEN], bf)
               for b in range(2)]
    a2a_in = nc.dram_tensor("a2a_in", [NCORES, DLOC, LT], bf)
    a2a_out = nc.dram_tensor("a2a_out", [NCORES, DLOC, LT], bf)

    a_dt = bf if OPT["a_bf16"] else f32

    with TileContext(nc) as tc:
        with (
            tc.tile_pool(name="pw", bufs=1) as pw,
            tc.tile_pool(name="pbig", bufs=1) as pbig,
            tc.tile_pool(name="ps1", bufs=1) as ps1,
            tc.tile_pool(name="psc", bufs=2) as psc,
            tc.tile_pool(name="pout", bufs=1) as pout,
            tc.tile_pool(name="ppa", bufs=4, space="PSUM") as ppa,
            tc.tile_pool(name="ppy", bufs=1, space="PSUM") as ppy,
        ):
            # ---- persistent weights / per-partition scalars ----
            wx = []
            wz = []
            for k in range(8):
                t = pw.tile([128, DLOC], bf, tag=f"wx{k}", name=f"wx{k}")
                nc.sync.dma_start(t[:], wxT[128 * k:128 * (k + 1), :])
                wx.append(t)
                t = pw.tile([128, DLOC], bf, tag=f"wz{k}", name=f"wz{k}")
                nc.sync.dma_start(t[:], wzT[128 * k:128 * (k + 1), :])
                wz.append(t)
            wxp = []
            for i in range(2):
                t = pw.tile([128, 33], bf, tag=f"wxp{i}", name=f"wxp{i}")
                nc.sync.dma_start(t[:], wpT[128 * i:128 * (i + 1), :])
                wxp.append(t)
            ident = pw.tile([128, 128], bf, tag="ident", name="ident")
            nc.sync.dma_start(ident[:], ident_d[:])
            cd = []
            for j in range(NRT * D_CONV):
                t = pw.tile([128, 128], bf, tag=f"cd{j}", name=f"cd{j}")
                nc.sync.dma_start(t[:], convd[128 * j:128 * (j + 1), :])
                cd.append(t)
            dsk = []
            for rt in range(NRT):
                t = pw.tile([128, 128], bf, tag=f"dsk{rt}", name=f"dsk{rt}")
                nc.sync.dma_start(t[:], dskd[128 * rt:128 * (rt + 1), :])
                dsk.append(t)

            cb, wd, bd, At = [], [], [], []
            for rt in range(NRT):
                rs = slice(128 * rt, 128 * (rt + 1))
                t = pw.tile([128, 1], f32, tag=f"cb{rt}", name=f"cb{rt}")
                nc.sync.dma_start(t[:], convb[rs, :])
                cb.append(t)
                t = pw.tile([128, 1], f32, tag=f"wd{rt}", name=f"wd{rt}")
                nc.sync.dma_start(t[:], wdt[rs, :])
                wd.append(t)
                t = pw.tile([128, 1], f32, tag=f"bd{rt}", name=f"bd{rt}")
                nc.sync.dma_start(t[:], bdt[rs, :])
                bd.append(t)
                ta = pw.tile([128, D_STATE], f32, tag=f"alog{rt}", name=f"alog{rt}")
                nc.sync.dma_start(ta[:], alog[rs, :])
                te = pw.tile([128, D_STATE], f32, tag=f"aexp{rt}", name=f"aexp{rt}")
                nc.scalar.activation(te[:], ta[:], AF.Exp)
                tn = pw.tile([128, D_STATE], f32, tag=f"aneg{rt}", name=f"aneg{rt}")
                nc.vector.tensor_scalar(tn[:], te[:], -1.0, None, OP.mult)
                At.append(tn)

            # ---- persistent activations (bufs=1: cross-rep reuse serializes
            # on last consumer, which is late in the scan phase anyway) ----
            x = [pbig.tile([128, SEQLEN], bf, tag=f"x{rt}", name=f"x{rt}")
                 for rt in range(NRT)]
            z = [pbig.tile([128, SEQLEN], bf, tag=f"z{rt}", name=f"z{rt}")
                 for rt in range(NRT)]
            u = [pbig.tile([128, SEQLEN], bf, tag=f"u{rt}", name=f"u{rt}")
                 for rt in range(NRT)]

            for _rep in range(reps):
                # ============ S1: in_proj + conv(+silu) + silu(z) ============
                x_pre = [
                    ps1.tile([128, SEQLEN + 3], bf, tag=f"xp{rt}", name=f"xp{rt}",
                             bufs=1)
                    for rt in range(NRT)
                ]
                for rt in range(NRT):
                    nc.vector.memset(x_pre[rt][:, 0:3], 0.0)
                for b in range(2):
                    for c4 in range(4):
                        cs = slice(b * SEQLEN + LT * c4, b * SEQLEN + LT * (c4 + 1))
                        ht = []
                        for k in range(8):
                            t = ps1.tile([128, LT], bf, tag=f"hid{k}",
                                         name=f"hid{k}", bufs=OPT["ht_bufs"])
                            nc.sync.dma_start(t[:], hidT[128 * k:128 * (k + 1), cs])
                            ht.append(t)
                        for i in range(2):
                            rt = 2 * b + i
                            ms = slice(128 * i, 128 * (i + 1))
                            px = ppa.tile([128, LT], f32, tag="pa", name="px")
                            for k in range(8):
                                nc.tensor.matmul(px[:], wx[k][:, ms], ht[k][:],
                                                 start=(k == 0), stop=(k == 7))
                            t0 = LT * c4
                            nc.scalar.copy(x_pre[rt][:, 3 + t0:3 + t0 + LT], px[:])
                            pz = ppa.tile([128, LT], f32, tag="pa", name="pz")
                            for k in range(8):
                                nc.tensor.matmul(pz[:], wz[k][:, ms], ht[k][:],
                                                 start=(k == 0), stop=(k == 7))
                            nc.scalar.activation(
                                z[rt][:, t0:t0 + LT], pz[:], AF.Silu)
                # conv: 4 accumulating diag-matmuls per (rt, 512-chunk)
                for rt in range(NRT):
                    for c4 in range(4):
                        pc = ppa.tile([128, LT], f32, tag="pa", name="pc")
                        for k in range(D_CONV):
                            nc.tensor.matmul(
                                pc[:], cd[rt * D_CONV + k][:],
                                x_pre[rt][:, LT * c4 + k:LT * c4 + k + LT],
                                start=(k == 0), stop=(k == 3))
                        nc.scalar.activation(
                            x[rt][:, LT * c4:LT * (c4 + 1)], pc[:], AF.Silu,
                            bias=cb[rt][:])

                # ============ S2: x_proj + AllReduce + dt/softplus ============
                dt_ = {}
                for b in range(2):
                    xdsb = psc.tile([33, SEQLEN], ar_dt, tag="xdsb",
                                    name=f"xdsb{b}", bufs=1)
                    for c4 in range(4):
                        ls = slice(LT * c4, LT * (c4 + 1))
                        p33 = ppa.tile([33, LT], f32, tag="pa", name="p33")
                        nc.tensor.matmul(p33[:], wxp[0][:], x[2 * b][:, ls],
                                         start=True, stop=False)
                        nc.tensor.matmul(p33[:], wxp[1][:], x[2 * b + 1][:, ls],
                                         start=False, stop=True)
                        nc.scalar.copy(xdsb[:, ls], p33[:])
                    nc.sync.dma_start(xdbl_part[b][:], xdsb[:])
                    nc.gpsimd.collective_compute(
                        "AllReduce", OP.add,
                        ins=[xdbl_part[b][:]], outs=[xdbl_full[b][:]],
                        replica_groups=[list(range(NCORES))],
                    )
                    if not OPT["ar_bf16"]:
                        xdsf = psc.tile([33, SEQLEN], f32, tag="xdsf",
                                        name=f"xdsf{b}", bufs=1)
                        nc.sync.dma_start(xdsf[:], xdbl_full[b][:])
                        bcr = psc.tile([33, SEQLEN], bf, tag="bcr",
                                       name=f"bcr{b}", bufs=1)
                        nc.vector.tensor_copy(bcr[:], xdsf[:, :])
                        nc.sync.dma_start(xdbl_bf[b][:], bcr[:])

                    # dt = softplus(W_dt*dt_raw + b_dt); u = dt*x
                    dtb_t = psc.tile([128, SEQLEN], ar_dt, tag="dtraw",
                                     name=f"dtraw{b}", bufs=1)
                    nc.sync.dma_start(
                        dtb_t[:],
                        xdbl_full[b][0:1, :].to_broadcast((128, SEQLEN)))
                    for i in range(2):
                        rt = 2 * b + i
                        dtt = pbig.tile([128, SEQLEN], f32, tag="dtp",
                                        name=f"dt{rt}", bufs=2)
                        # softplus(v) = ln(exp(v) + 1), exp then in-place ln
                        nc.scalar.activation(dtt[:], dtb_t[:], AF.Exp,
                                             scale=wd[rt][:], bias=bd[rt][:])
                        nc.scalar.activation(dtt[:], dtt[:], AF.Ln, bias=1.0)
                        dt_[rt] = dtt
                        nc.vector.tensor_tensor(u[rt][:], dtt[:], x[rt][:],
                                                OP.mult)

                # ============ S3: selective scan + gating (i-major) ============
                bsrc = xdbl_full if OPT["ar_bf16"] else xdbl_bf
                for b in range(2):
                    for i in range(2):
                        rt = 2 * b + i
                        py = ppy.tile([128, SEQLEN], f32, tag="py", name=f"py{rt}")
                        # D_skip term opens each accumulation group (so x's
                        # last consumer is early, not at scan end)
                        for q in range(4):
                            nc.tensor.matmul(
                                py[:, LT * q:LT * (q + 1)],
                                dsk[rt][:],
                                x[rt][:, LT * q:LT * (q + 1)],
                                start=True, stop=False)
                        for n in range(16):
                            Bb = psc.tile([128, SEQLEN], bf, tag="Bb", name="Bb",
                                          bufs=2)
                            nc.gpsimd.dma_start(
                                Bb[:],
                                bsrc[b][1 + n:2 + n, :].to_broadcast(
                                    (128, SEQLEN)))
                            Cb = psc.tile([128, SEQLEN], bf, tag="Cb", name="Cb",
                                          bufs=2)
                            nc.gpsimd.dma_start(
                                Cb[:],
                                bsrc[b][17 + n:18 + n, :].to_broadcast(
                                    (128, SEQLEN)))
                            a_t = psc.tile([128, SEQLEN], a_dt, tag="a",
                                           name="a", bufs=2)
                            nc.scalar.activation(
                                a_t[:], dt_[rt][:], AF.Exp,
                                scale=At[rt][:, n:n + 1])
                            bb = psc.tile([128, SEQLEN], bf, tag="bb",
                                          name="bb", bufs=2)
                            if n < OPT["bb_pool"]:
                                nc.gpsimd.tensor_tensor(
                                    bb[:], u[rt][:], Bb[:], OP.mult)
                            else:
                                nc.vector.tensor_tensor(
                                    bb[:], u[rt][:], Bb[:], OP.mult)
                            hh = psc.tile([128, SEQLEN], bf, tag="hh",
                                          name="hh", bufs=2)
                            nc.vector.tensor_tensor_scan(
                                hh[:], a_t[:], bb[:], 0.0,
                                OP.mult, OP.add)
                            ww = psc.tile([128, SEQLEN], bf, tag="ww",
                                          name="ww", bufs=2)
                            if n < OPT["ww_pool"]:
                                nc.gpsimd.tensor_tensor(
                                    ww[:], hh[:], Cb[:], OP.mult)
                            else:
                                nc.vector.tensor_tensor(
                                    ww[:], hh[:], Cb[:], OP.mult)
                            for q in range(4):
                                nc.tensor.matmul(
                                    py[:, LT * q:LT * (q + 1)],
                                    ident[:],
                                    ww[:, LT * q:LT * (q + 1)],
                                    start=False, stop=(n == 15))
                        # gating
                        for c2 in range(2):
                            gs = slice(LC * c2, LC * (c2 + 1))
                            ysc = psc.tile([128, LC], bf, tag="ysc",
                                           name="ysc", bufs=1)
                            nc.scalar.copy(ysc[:], py[:, gs])
                            yg = psc.tile([128, LC], bf, tag="yg",
                                          name="yg", bufs=1)
                            nc.vector.tensor_tensor(
                                yg[:], ysc[:], z[rt][:, gs], OP.mult)
                            for hf in range(2):
                                j = 4 * b + 2 * c2 + hf
                                nc.sync.dma_start(
                                    a2a_in[j, 128 * i:128 * (i + 1), :],
                                    yg[:, LT * hf:LT * (hf + 1)])

                # ================= S8: AllToAll + out_proj =================
                nc.gpsimd.collective_compute(
                    "AllToAll", OP.bypass,
                    ins=[a2a_in[:]], outs=[a2a_out[:]],
                    replica_groups=[list(range(NCORES))],
                )
                yf = []
                for kt in range(16):
                    t = pout.tile([128, LT], bf, tag=f"yf{kt}", name=f"yf{kt}")
                    nc.sync.dma_start(
                        t[:], a2a_out[kt // 2, 128 * (kt % 2):128 * (kt % 2 + 1), :])
                    yf.append(t)
                for g in range(2):
                    wo = []
                    for kt in range(16):
                        t = pout.tile([128, LT], bf, tag=f"wo{kt}",
                                      name=f"wo{kt}", bufs=1)
                        nc.sync.dma_start(
                            t[:], woT[128 * kt:128 * (kt + 1),
                                      LT * g:LT * (g + 1)])
                        wo.append(t)
                    for mh in range(4):
                        mt = 4 * g + mh
                        pO = ppa.tile([128, LT], f32, tag="pa", name="pO")
                        for kt in range(16):
                            nc.tensor.matmul(
                                pO[:], wo[kt][:, 128 * mh:128 * (mh + 1)],
                                yf[kt][:], start=(kt == 0), stop=(kt == 15))
                        osb = psc.tile([128, LT], f32, tag="osb", name="osb",
                                       bufs=2)
                        nc.scalar.copy(osb[:], pO[:])
                        nc.sync.dma_start(out_loc[128 * mt:128 * (mt + 1), :],
                                          osb[:])

    _split_sync_waits(nc)
    return nc


def _prep_inputs(inputs):
    hidden = np.ascontiguousarray(inputs["hidden_states"], np.float32)
    W_in = np.asarray(inputs["W_in"], np.float32)
    conv_w = np.asarray(inputs["conv_w"], np.float32)
    conv_b = np.asarray(inputs["conv_b"], np.float32)
    W_xproj = np.asarray(inputs["W_xproj"], np.float32)
    W_dt = np.asarray(inputs["W_dt"], np.float32)
    b_dt = np.asarray(inputs["b_dt"], np.float32)
    A_log = np.asarray(inputs["A_log"], np.float32)
    D_skip = np.asarray(inputs["D_skip"], np.float32)
    W_out = np.asarray(inputs["W_out"], np.float32)

    # hidden^T: [d_model, b*L]
    hidT = np.ascontiguousarray(
        hidden.transpose(2, 0, 1).reshape(D_MODEL, TOK)).astype(BF16)
    woT = np.ascontiguousarray(W_out.T).astype(BF16)
    ident = np.eye(128, dtype=np.float32).astype(BF16)

    in_maps = []
    for c in range(NCORES):
        dsl = slice(DLOC * c, DLOC * (c + 1))
        dup = lambda v: np.ascontiguousarray(
            np.tile(v[dsl].reshape(DLOC, -1), (BATCH, 1)), ).astype(np.float32)
        cwl = dup(conv_w[:, 0, :])                      # [ROWS, 4]
        convd = np.zeros((NRT * D_CONV, 128, 128), np.float32)
        for rt in range(NRT):
            for k in range(D_CONV):
                np.fill_diagonal(convd[rt * D_CONV + k],
                                 cwl[128 * rt:128 * (rt + 1), k])
        dskl = dup(D_skip)                              # [ROWS, 1]
        dskd = np.zeros((NRT, 128, 128), np.float32)
        for rt in range(NRT):
            np.fill_diagonal(dskd[rt], dskl[128 * rt:128 * (rt + 1), 0])
        in_maps.append(dict(
            hidT=hidT,
            wxT=np.ascontiguousarray(W_in[dsl, :].T).astype(BF16),
            wzT=np.ascontiguousarray(W_in[D_INNER:, :][dsl, :].T).astype(BF16),
            wpT=np.ascontiguousarray(W_xproj[:, dsl].T).astype(BF16),
            woT=woT,
            convd=np.ascontiguousarray(
                convd.reshape(NRT * D_CONV * 128, 128)).astype(BF16),
            dskd=np.ascontiguousarray(
                dskd.reshape(NRT * 128, 128)).astype(BF16),
            convb=dup(conv_b),
            wdt=dup(W_dt),
            bdt=dup(b_dt),
            alog=dup(A_log),
            ident=ident,
        ))
    return in_maps


def _gather(results):
    out = np.empty((BATCH, SEQLEN, D_MODEL), np.float32)
    for c in range(NCORES):
        b, lc = c // 4, c % 4
        out[b, LT * lc:LT * (lc + 1), :] = results[c]["out_loc"].T
    return out


def _get_nc(reps=None):
    key = ("nc", REPEATS if reps is None else reps, _opt_key())
    if key not in _cache:
        _cache[key] = _build_nc(repeats=reps)
    return _cache[key]


def _get_runner(reps=None, donate_outs=True):
    key = ("runner", REPEATS if reps is None else reps, donate_outs, _opt_key())
    nc = _get_nc(reps)
    if key not in _cache:
        from concourse import bass2jax
        import jax
        import concourse.mybir as mybir
        from jax.sharding import Mesh, PartitionSpec
        from jax.experimental.shard_map import shard_map

        bass2jax.install_neuronx_cc_hook()
        partition_name = (nc.partition_id_tensor.name
                          if nc.partition_id_tensor else None)
        in_names, out_names, out_avals, zero_outs = [], [], [], []
        for alloc in nc.m.functions[0].allocations:
            if not isinstance(alloc, mybir.MemoryLocationSet):
                continue
            name = alloc.memorylocations[0].name
            if alloc.kind == "ExternalInput":
                if name != partition_name:
                    in_names.append(name)
            elif alloc.kind == "ExternalOutput":
                out_names.append(name)
                shape = tuple(alloc.tensor_shape)
                dtype = mybir.dt.np(alloc.dtype)
                out_avals.append(jax.core.ShapedArray(shape, dtype))
                zero_outs.append(np.zeros(shape, dtype))
        n_params = len(in_names)
        all_in = in_names + out_names
        if partition_name is not None:
            all_in = all_in + [partition_name]
        donate = tuple(range(n_params, n_params + len(out_names)))

        def _body(*args):
            operands = list(args)
            if partition_name is not None:
                operands.append(bass2jax.partition_id_tensor())
            outs = bass2jax._bass_exec_p.bind(
                *operands,
                out_avals=tuple(out_avals),
                in_names=tuple(all_in),
                out_names=tuple(out_names),
                lowering_input_output_aliases=(),
                sim_require_finite=True,
                sim_require_nnan=True,
                nc=nc,
            )
            return tuple(outs)

        devices = jax.devices()[:NCORES]
        mesh = Mesh(np.asarray(devices), ("core",))
        in_specs = (PartitionSpec("core"),) * (n_params + len(out_names))
        out_specs = (PartitionSpec("core"),) * len(out_names)
        sharded = jax.jit(
            shard_map(_body, mesh=mesh, in_specs=in_specs, out_specs=out_specs,
                      check_rep=False),
            donate_argnums=(donate if donate_outs else ()), keep_unused=True)
        _cache[key] = (sharded, in_names, out_names, zero_outs, n_params, mesh)
    return _cache[key]


def device_put_inputs(in_maps, reps=None, donate_outs=True):
    """Concatenate per-core inputs and place them sharded on the devices."""
    import jax
    from jax.sharding import NamedSharding, PartitionSpec

    sharded, in_names, out_names, zero_outs, n_params, mesh = _get_runner(
        reps, donate_outs)
    sh = NamedSharding(mesh, PartitionSpec("core"))
    concat_in = [
        np.concatenate([np.asarray(in_maps[c][nm]) for c in range(NCORES)], axis=0)
        for nm in in_names
    ] + [np.concatenate([z] * NCORES, axis=0) for z in zero_outs]
    return [jax.device_put(a, sh) for a in concat_in]


def run_spmd(in_maps=None, dev_in=None, reps=None, donate_outs=True,
             as_numpy=True):
    """Execute on cores 0-7. First call compiles; later calls reuse the
    jitted executable."""
    sharded, in_names, out_names, zero_outs, n_params, mesh = _get_runner(
        reps, donate_outs)
    if dev_in is None:
        dev_in = device_put_inputs(in_maps, reps, donate_outs)
    outs = sharded(*dev_in)
    if not as_numpy:
        return outs
    outs = [np.asarray(o) for o in outs]
    results = []
    for c in range(NCORES):
        d = {}
        for i, nm in enumerate(out_names):
            per = outs[i].shape[0] // NCORES
            d[nm] = outs[i][per * c:per * (c + 1)]
        results.append(d)
    return results


def kernel(**inputs) -> np.ndarray:
    in_maps = _prep_inputs(inputs)
    results = run_spmd(in_maps)
    return _gather(results)
